# revision 12
# baseline (speedup 1.0000x reference)
"""Bass kernel for nn_Attn_1898375545663 on 8 TRN2 NeuronCores.

Reference (single device):
    energies[b, l] = sum_h hidden[h, b] * encoder_outputs[l, b, h]   # [B, L]
    attn = softmax(energies, axis=1)                                 # [B, L]
    return attn[:, None, :]                                          # [B, 1, L]

Shapes: L=4096, B=32, H=1024. encoder_outputs is 512 MB fp32 -> memory bound.
Sharding: pure data parallel over batch; each of the 8 cores gets 4 batches,
no collectives.

vs the fp32 baseline (~200 us):

- The 2e-2 rel-err budget admits fp16 inputs (measured l2 rel err 1.9e-3,
  ~10x margin), halving HBM traffic to 32 MB/core -> ~94-100 us DMA floor.
- The custom-DVE reduce has no 16-bit fast path (would be DVE-bound at
  ~137 us), so the dot products run on the PE instead: the host
  pre-transposes each core's shard to [(b, g, ph), (t, l)] fp16 so every
  DMA is one contiguous 2 MB block ([128 partitions x 16 KB]); per
  (b, hc) chunk, 32 matmuls with stationary lhsT = enc[:, lt*128:+128]
  ([K=128 h, M=128 l]) and moving rhs = hid[:, hc*4:+4] ([128 h, 4 b])
  accumulate into a per-batch PSUM bank mm[128 l, 32*4]. fp16 stationary
  streams 1 col/cycle @ 2.4 GHz -> ~55 us PE busy, under the DMA floor.
- PSUM start=True marks the whole 2 KB bank pending-zero (writes to
  pending bytes store-and-clear, others accumulate), so each batch opens
  its bank with ONE full-width zeroing matmul and everything after
  accumulates with start=False; the full-width write also gives every
  later matmul a WAW dep on the reset (ordering).
- Softmax is per batch and overlapped: batch b's tail (max via PE
  transpose + ones-matmul broadcast instead of slow gpsimd
  partition_all_reduce, exp straight out of PSUM, PE row-sum, reciprocal,
  PE transpose, scale, 16 KB output DMA) runs while batch b+1 streams,
  so only the last batch's ~2 us tail is exposed.
"""

import numpy as np

from concourse import bacc, mybir, tile
from concourse.bass_utils import run_bass_kernel_spmd
from concourse.masks import make_identity

L, B, H = 4096, 32, 1024
NCORES = 8
BS = B // NCORES          # 4 batches per core
P = 128                   # partitions
HC = H // P               # 8 h-chunks per batch
NT = L // P               # 32 l-tiles per batch
TG = 2                    # h-chunks per DMA (2 MB per dma_start)
F32 = mybir.dt.float32
F16 = mybir.dt.float16

_cached = {}


def batch_block(nc, inp, enc, hid_sb, consts_t, mmps, small, work, out_ext,
                b, tg, queues, nd0):
    """Stream + accumulate one batch's energies, then its softmax tail."""
    iden, ones, ones_row, z16 = consts_t
    ngrp = HC // tg
    mm = mmps.tile([P, 512], F32)  # full PSUM bank (zero-region isolation)
    nc.tensor.matmul(mm[:, : NT * BS], z16[:], z16[:], start=True, stop=False)
    nd = nd0
    for g in range(ngrp):
        tile_in = inp.tile([P, tg * L], F16)
        r0 = (b * ngrp + g) * P
        eng = getattr(nc, queues[nd % len(queues)])
        nd += 1
        eng.dma_start(tile_in[:], enc[r0 : r0 + P, : tg * L])
        for t in range(tg):
            hc = g * tg + t
            for lt in range(NT):
                nc.tensor.matmul(
                    mm[:, lt * BS : (lt + 1) * BS],
                    tile_in[:, t * L + lt * P : t * L + (lt + 1) * P],
                    hid_sb[:, hc * BS : (hc + 1) * BS],
                    start=False,
                    stop=(hc == HC - 1 and lt == NT - 1),
                )
    # column b of each [P, BS] group is this batch's energies:
    # mmb[p, lt] = e(l = lt*128 + p)
    mmb = mm[:, : NT * BS].rearrange("p (lt four) -> p lt four", four=BS)[:, :, b]

    # ---- softmax tail (per-row max subtraction is exact) ----
    m1 = work.tile([P, 1], F32)
    nc.vector.tensor_reduce(out=m1[:], in_=mmb, axis=mybir.AxisListType.X,
                            op=mybir.AluOpType.max)
    tr = small.tile([1, P], F32)
    nc.tensor.transpose(tr[:], m1[:], iden[:])
    trs = work.tile([1, P], F32)
    nc.scalar.copy(trs[:], tr[:])
    mxs = work.tile([1, 1], F32)
    nc.vector.tensor_reduce(out=mxs[:], in_=trs[:], axis=mybir.AxisListType.X,
                            op=mybir.AluOpType.max)
    bc = small.tile([P, 1], F32)
    nc.tensor.matmul(bc[:], ones_row[:], mxs[:], start=True, stop=True)
    negm = work.tile([P, 1], F32)
    nc.scalar.mul(negm[:], bc[:], -1.0)

    p_b = work.tile([P, NT], F32)
    nc.scalar.activation(p_b[:], mmb, mybir.ActivationFunctionType.Exp,
                         bias=negm[:], scale=1.0)

    s_b = work.tile([P, 1], F32)
    nc.vector.tensor_reduce(out=s_b[:], in_=p_b[:], axis=mybir.AxisListType.X,
                            op=mybir.AluOpType.add)
    s_ps = small.tile([1, 1], F32)
    nc.tensor.matmul(s_ps[:], s_b[:], ones[:], start=True, stop=True)
    r_sb = work.tile([1, 1], F32)
    nc.vector.reciprocal(r_sb[:], s_ps[:])
    rb = small.tile([P, 1], F32)
    nc.tensor.matmul(rb[:], ones_row[:], r_sb[:], start=True, stop=True)
    rb_sb = work.tile([P, 1], F32)
    nc.scalar.copy(rb_sb[:], rb[:])

    tp = small.tile([NT, P], F32)
    nc.tensor.transpose(tp[:], p_b[:], iden[:])
    attn_b = work.tile([NT, P], F32)
    nc.vector.tensor_scalar(out=attn_b[:], in0=tp[:], scalar1=rb_sb[:NT, :],
                            scalar2=None, op0=mybir.AluOpType.mult)
    nc.scalar.dma_start(out_ext[b * NT : (b + 1) * NT, :], attn_b[:])
    return nd


def build_nc(repeat=1, use_for_i=False, tg=TG, inp_bufs=6,
             queues=("sync", "scalar")):
    nc = bacc.Bacc(trn_type="TRN2")

    enc = nc.declare_dram_parameter("enc", [BS * H // tg, tg * L], F16,
                                    isOutput=False)
    hid = nc.declare_dram_parameter("hid", [P, HC * BS], F16, isOutput=False)
    out_ext = nc.declare_dram_parameter("out", [BS * NT, P], F32, isOutput=True)

    with tile.TileContext(nc) as tc:
        with (
            tc.tile_pool(name="consts", bufs=1) as consts,
            tc.tile_pool(name="inp", bufs=inp_bufs) as inp,
            tc.tile_pool(name="work", bufs=2) as work,
            tc.tile_pool(name="mmps", bufs=2, space="PSUM") as mmps,
            tc.tile_pool(name="small", bufs=1, space="PSUM") as small,
        ):
            hid_sb = consts.tile([P, HC * BS], F16)
            nc.sync.dma_start(hid_sb[:], hid[:])
            iden = consts.tile([P, P], F32)
            make_identity(nc, iden[:])
            ones = consts.tile([P, 1], F32)
            nc.gpsimd.memset(ones[:], 1.0)
            ones_row = consts.tile([1, P], F32)
            nc.gpsimd.memset(ones_row[:], 1.0)
            z16 = consts.tile([P, NT * BS], F16)
            nc.gpsimd.memset(z16[:], 0.0)

            consts_t = (iden, ones, ones_row, z16)

            def body():
                nd = 0
                for b in range(BS):
                    nd = batch_block(nc, inp, enc, hid_sb, consts_t, mmps,
                                     small, work, out_ext, b, tg, queues, nd)

            if use_for_i and repeat > 1:
                with tc.For_i(0, repeat, 1):
                    body()
            else:
                for _rep in range(repeat):
                    body()

    nc.compile()
    return nc


def make_in_maps(hidden, encoder_outputs, tg=TG):
    hidden = np.asarray(hidden)
    enc = np.asarray(encoder_outputs)
    assert hidden.shape == (H, B) and enc.shape == (L, B, H)

    enc16 = enc.astype(np.float16)
    hid16 = hidden.astype(np.float16)
    ngrp = HC // tg

    in_maps = []
    for c in range(NCORES):
        bsl = slice(c * BS, (c + 1) * BS)
        # [L, BS, H] -> [b, h, l] -> [b, g, p, t, l]: one contiguous
        # [P, tg*L] block per DMA, tg*8KB per-partition lines
        enc_t = np.ascontiguousarray(
            enc16[:, bsl, :]
            .transpose(1, 2, 0)
            .reshape(BS, ngrp, tg, P, L)
            .transpose(0, 1, 3, 2, 4)
        ).reshape(BS * H // tg, tg * L)
        # [H, BS] -> [hc, ph, b] -> [ph, hc*BS + b]
        hid_t = np.ascontiguousarray(
            hid16[:, bsl].reshape(HC, P, BS).transpose(1, 0, 2)
        ).reshape(P, HC * BS)
        in_maps.append({"enc": enc_t, "hid": hid_t})
    return in_maps


def _get_nc():
    if "nc" not in _cached:
        _cached["nc"] = build_nc()
    return _cached["nc"]


def kernel(hidden, encoder_outputs, **kwargs):
    in_maps = make_in_maps(hidden, encoder_outputs)
    nc = _get_nc()
    res = run_bass_kernel_spmd(nc, in_maps, core_ids=list(range(NCORES)))
    outs = [res.results[i]["out"].reshape(BS, 1, L) for i in range(NCORES)]
    return np.concatenate(outs, axis=0)


# revision 13
# speedup vs baseline: 1.0209x; 1.0209x over previous
"""Bass kernel for nn_Attn_1898375545663 on 8 TRN2 NeuronCores.

Reference (single device):
    energies[b, l] = sum_h hidden[h, b] * encoder_outputs[l, b, h]   # [B, L]
    attn = softmax(energies, axis=1)                                 # [B, L]
    return attn[:, None, :]                                          # [B, 1, L]

Shapes: L=4096, B=32, H=1024. encoder_outputs is 512 MB fp32 -> memory bound.
Sharding: pure data parallel over batch; each of the 8 cores gets 4 batches,
no collectives.

vs the fp32 baseline (~200 us):

- The 2e-2 rel-err budget admits fp16 inputs (measured l2 rel err 1.9e-3,
  ~10x margin), halving HBM traffic to 32 MB/core -> ~94-100 us DMA floor.
- The custom-DVE reduce has no 16-bit fast path (would be DVE-bound at
  ~137 us), so the dot products run on the PE instead: the host
  pre-transposes each core's shard to [(b, g, ph), (t, l)] fp16 so every
  DMA is one contiguous 1 MB block ([128 partitions x 8 KB]); per
  (b, hc) chunk, 32 matmuls with stationary lhsT = enc[:, lt*128:+128]
  ([K=128 h, M=128 l]) and moving rhs = hid[:, hc*4:+4] ([128 h, 4 b])
  accumulate into a per-batch PSUM bank mm[128 l, 32*4]. fp16 stationary
  streams 1 col/cycle @ 2.4 GHz -> ~55 us PE busy, under the DMA floor.
- PSUM start=True marks the whole 2 KB bank pending-zero (writes to
  pending bytes store-and-clear, others accumulate), so each batch opens
  its bank with ONE full-width zeroing matmul and everything after
  accumulates with start=False; the full-width write also gives every
  later matmul a WAW dep on the reset (ordering).
- Softmax is per batch and overlapped: batch b's tail (max via PE
  transpose + ones-matmul broadcast instead of slow gpsimd
  partition_all_reduce, exp straight out of PSUM, PE row-sum, reciprocal,
  PE transpose, scale, 16 KB output DMA) runs while batch b+1 streams,
  so only the last batch's ~2 us tail is exposed.
"""

import numpy as np

from concourse import bacc, mybir, tile
from concourse.bass_utils import run_bass_kernel_spmd
from concourse.masks import make_identity

L, B, H = 4096, 32, 1024
NCORES = 8
BS = B // NCORES          # 4 batches per core
P = 128                   # partitions
HC = H // P               # 8 h-chunks per batch
NT = L // P               # 32 l-tiles per batch
TG = 1                    # h-chunks per DMA (1 MB per dma_start)
F32 = mybir.dt.float32
F16 = mybir.dt.float16

_cached = {}


def batch_block(nc, inp, enc, hid_sb, consts_t, mmps, small, work, out_ext,
                b, tg, queues, nd0):
    """Stream + accumulate one batch's energies, then its softmax tail."""
    iden, ones, ones_row, z16 = consts_t
    ngrp = HC // tg
    mm = mmps.tile([P, 512], F32)  # full PSUM bank (zero-region isolation)
    nc.tensor.matmul(mm[:, : NT * BS], z16[:], z16[:], start=True, stop=False)
    nd = nd0
    for g in range(ngrp):
        tile_in = inp.tile([P, tg * L], F16)
        r0 = (b * ngrp + g) * P
        eng = getattr(nc, queues[nd % len(queues)])
        nd += 1
        eng.dma_start(tile_in[:], enc[r0 : r0 + P, : tg * L])
        for t in range(tg):
            hc = g * tg + t
            for lt in range(NT):
                nc.tensor.matmul(
                    mm[:, lt * BS : (lt + 1) * BS],
                    tile_in[:, t * L + lt * P : t * L + (lt + 1) * P],
                    hid_sb[:, hc * BS : (hc + 1) * BS],
                    start=False,
                    stop=(hc == HC - 1 and lt == NT - 1),
                )
    # column b of each [P, BS] group is this batch's energies:
    # mmb[p, lt] = e(l = lt*128 + p)
    mmb = mm[:, : NT * BS].rearrange("p (lt four) -> p lt four", four=BS)[:, :, b]

    # ---- softmax tail (per-row max subtraction is exact) ----
    m1 = work.tile([P, 1], F32)
    nc.vector.tensor_reduce(out=m1[:], in_=mmb, axis=mybir.AxisListType.X,
                            op=mybir.AluOpType.max)
    tr = small.tile([1, P], F32)
    nc.tensor.transpose(tr[:], m1[:], iden[:])
    trs = work.tile([1, P], F32)
    nc.scalar.copy(trs[:], tr[:])
    mxs = work.tile([1, 1], F32)
    nc.vector.tensor_reduce(out=mxs[:], in_=trs[:], axis=mybir.AxisListType.X,
                            op=mybir.AluOpType.max)
    bc = small.tile([P, 1], F32)
    nc.tensor.matmul(bc[:], ones_row[:], mxs[:], start=True, stop=True)
    negm = work.tile([P, 1], F32)
    nc.scalar.mul(negm[:], bc[:], -1.0)

    p_b = work.tile([P, NT], F32)
    nc.scalar.activation(p_b[:], mmb, mybir.ActivationFunctionType.Exp,
                         bias=negm[:], scale=1.0)

    s_b = work.tile([P, 1], F32)
    nc.vector.tensor_reduce(out=s_b[:], in_=p_b[:], axis=mybir.AxisListType.X,
                            op=mybir.AluOpType.add)
    s_ps = small.tile([1, 1], F32)
    nc.tensor.matmul(s_ps[:], s_b[:], ones[:], start=True, stop=True)
    r_sb = work.tile([1, 1], F32)
    nc.vector.reciprocal(r_sb[:], s_ps[:])
    rb = small.tile([P, 1], F32)
    nc.tensor.matmul(rb[:], ones_row[:], r_sb[:], start=True, stop=True)
    rb_sb = work.tile([P, 1], F32)
    nc.scalar.copy(rb_sb[:], rb[:])

    tp = small.tile([NT, P], F32)
    nc.tensor.transpose(tp[:], p_b[:], iden[:])
    attn_b = work.tile([NT, P], F32)
    nc.vector.tensor_scalar(out=attn_b[:], in0=tp[:], scalar1=rb_sb[:NT, :],
                            scalar2=None, op0=mybir.AluOpType.mult)
    nc.scalar.dma_start(out_ext[b * NT : (b + 1) * NT, :], attn_b[:])
    return nd


def build_nc(repeat=1, use_for_i=False, tg=TG, inp_bufs=10,
             queues=("sync", "scalar")):
    nc = bacc.Bacc(trn_type="TRN2")

    enc = nc.declare_dram_parameter("enc", [BS * H // tg, tg * L], F16,
                                    isOutput=False)
    hid = nc.declare_dram_parameter("hid", [P, HC * BS], F16, isOutput=False)
    out_ext = nc.declare_dram_parameter("out", [BS * NT, P], F32, isOutput=True)

    with tile.TileContext(nc) as tc:
        with (
            tc.tile_pool(name="consts", bufs=1) as consts,
            tc.tile_pool(name="inp", bufs=inp_bufs) as inp,
            tc.tile_pool(name="work", bufs=2) as work,
            tc.tile_pool(name="mmps", bufs=2, space="PSUM") as mmps,
            tc.tile_pool(name="small", bufs=1, space="PSUM") as small,
        ):
            hid_sb = consts.tile([P, HC * BS], F16)
            nc.sync.dma_start(hid_sb[:], hid[:])
            iden = consts.tile([P, P], F32)
            make_identity(nc, iden[:])
            ones = consts.tile([P, 1], F32)
            nc.gpsimd.memset(ones[:], 1.0)
            ones_row = consts.tile([1, P], F32)
            nc.gpsimd.memset(ones_row[:], 1.0)
            z16 = consts.tile([P, NT * BS], F16)
            nc.gpsimd.memset(z16[:], 0.0)

            consts_t = (iden, ones, ones_row, z16)

            def body():
                nd = 0
                for b in range(BS):
                    nd = batch_block(nc, inp, enc, hid_sb, consts_t, mmps,
                                     small, work, out_ext, b, tg, queues, nd)

            if use_for_i and repeat > 1:
                with tc.For_i(0, repeat, 1):
                    body()
            else:
                for _rep in range(repeat):
                    body()

    nc.compile()
    return nc


def make_in_maps(hidden, encoder_outputs, tg=TG):
    hidden = np.asarray(hidden)
    enc = np.asarray(encoder_outputs)
    assert hidden.shape == (H, B) and enc.shape == (L, B, H)

    enc16 = enc.astype(np.float16)
    hid16 = hidden.astype(np.float16)
    ngrp = HC // tg

    in_maps = []
    for c in range(NCORES):
        bsl = slice(c * BS, (c + 1) * BS)
        # [L, BS, H] -> [b, h, l] -> [b, g, p, t, l]: one contiguous
        # [P, tg*L] block per DMA, tg*8KB per-partition lines
        enc_t = np.ascontiguousarray(
            enc16[:, bsl, :]
            .transpose(1, 2, 0)
            .reshape(BS, ngrp, tg, P, L)
            .transpose(0, 1, 3, 2, 4)
        ).reshape(BS * H // tg, tg * L)
        # [H, BS] -> [hc, ph, b] -> [ph, hc*BS + b]
        hid_t = np.ascontiguousarray(
            hid16[:, bsl].reshape(HC, P, BS).transpose(1, 0, 2)
        ).reshape(P, HC * BS)
        in_maps.append({"enc": enc_t, "hid": hid_t})
    return in_maps


def _get_nc():
    if "nc" not in _cached:
        _cached["nc"] = build_nc()
    return _cached["nc"]


def kernel(hidden, encoder_outputs, **kwargs):
    in_maps = make_in_maps(hidden, encoder_outputs)
    nc = _get_nc()
    res = run_bass_kernel_spmd(nc, in_maps, core_ids=list(range(NCORES)))
    outs = [res.results[i]["out"].reshape(BS, 1, L) for i in range(NCORES)]
    return np.concatenate(outs, axis=0)


# revision 14
# speedup vs baseline: 1.0294x; 1.0083x over previous
"""Bass kernel for nn_Attn_1898375545663 on 8 TRN2 NeuronCores.

Reference (single device):
    energies[b, l] = sum_h hidden[h, b] * encoder_outputs[l, b, h]   # [B, L]
    attn = softmax(energies, axis=1)                                 # [B, L]
    return attn[:, None, :]                                          # [B, 1, L]

Shapes: L=4096, B=32, H=1024. encoder_outputs is 512 MB fp32 -> memory bound.
Sharding: pure data parallel over batch; each of the 8 cores gets 4 batches,
no collectives.

vs the fp32 baseline (~200 us):

- The 2e-2 rel-err budget admits fp16 inputs (measured l2 rel err 1.9e-3,
  ~10x margin), halving HBM traffic to 32 MB/core -> ~94-100 us DMA floor.
- The custom-DVE reduce has no 16-bit fast path (would be DVE-bound at
  ~137 us), so the dot products run on the PE instead: the host
  pre-transposes each core's shard to [(b, g, ph), (t, l)] fp16 so every
  DMA is one contiguous 1 MB block ([128 partitions x 8 KB]); per
  (b, hc) chunk, 32 matmuls with stationary lhsT = enc[:, lt*128:+128]
  ([K=128 h, M=128 l]) and moving rhs = hid[:, hc*4:+4] ([128 h, 4 b])
  accumulate into a per-batch PSUM bank mm[128 l, 32*4]. fp16 stationary
  streams 1 col/cycle @ 2.4 GHz -> ~55 us PE busy, under the DMA floor.
- PSUM start=True marks the whole 2 KB bank pending-zero (writes to
  pending bytes store-and-clear, others accumulate), so each batch opens
  its bank with ONE full-width zeroing matmul and everything after
  accumulates with start=False; the full-width write also gives every
  later matmul a WAW dep on the reset (ordering).
- The per-batch softmax tail keeps the PE untouched (any PE op in the tail
  stalls the in-order PE stream behind DVE/ACT dependencies): max/sum
  partition reductions run on the otherwise-idle gpsimd
  (partition_all_reduce), exp reads the PSUM accumulator directly, the
  1/sum scale is one DVE tensor_scalar, and the [128 p, 32 lt] block DMAs
  out untransposed -- the host does the free 16 KB reorder. Tails for
  batches 0..2 hide under the next batch's stream; only the last ~3 us is
  exposed. mmps rotates over 4 full banks so bank reuse never couples the
  PE stream to an older batch's tail.
"""

import numpy as np

from concourse import bacc, mybir, tile
from concourse.bass_isa import ReduceOp
from concourse.bass_utils import run_bass_kernel_spmd

L, B, H = 4096, 32, 1024
NCORES = 8
BS = B // NCORES          # 4 batches per core
P = 128                   # partitions
HC = H // P               # 8 h-chunks per batch
NT = L // P               # 32 l-tiles per batch
TG = 1                    # h-chunks per DMA (1 MB per dma_start)
F32 = mybir.dt.float32
F16 = mybir.dt.float16

_cached = {}


def batch_block(nc, inp, enc, hid_sb, z16, mmps, work, out_ext,
                b, tg, queues, nd0):
    """Stream + accumulate one batch's energies, then its softmax tail."""
    ngrp = HC // tg
    mm = mmps.tile([P, 512], F32)  # full PSUM bank (zero-region isolation)
    nc.tensor.matmul(mm[:, : NT * BS], z16[:], z16[:], start=True, stop=False)
    nd = nd0
    for g in range(ngrp):
        tile_in = inp.tile([P, tg * L], F16)
        r0 = (b * ngrp + g) * P
        eng = getattr(nc, queues[nd % len(queues)])
        nd += 1
        eng.dma_start(tile_in[:], enc[r0 : r0 + P, : tg * L])
        for t in range(tg):
            hc = g * tg + t
            for lt in range(NT):
                nc.tensor.matmul(
                    mm[:, lt * BS : (lt + 1) * BS],
                    tile_in[:, t * L + lt * P : t * L + (lt + 1) * P],
                    hid_sb[:, hc * BS : (hc + 1) * BS],
                    start=False,
                    stop=(hc == HC - 1 and lt == NT - 1),
                )
    # column b of each [P, BS] group is this batch's energies:
    # mmb[p, lt] = e(l = lt*128 + p)
    mmb = mm[:, : NT * BS].rearrange("p (lt four) -> p lt four", four=BS)[:, :, b]

    # ---- softmax tail, PE-free (per-row max subtraction is exact) ----
    m1 = work.tile([P, 1], F32)
    nc.vector.tensor_reduce(out=m1[:], in_=mmb, axis=mybir.AxisListType.X,
                            op=mybir.AluOpType.max)
    mx = work.tile([P, 1], F32)
    nc.gpsimd.partition_all_reduce(mx[:], m1[:], P, ReduceOp.max)
    negm = work.tile([P, 1], F32)
    nc.scalar.mul(negm[:], mx[:], -1.0)

    p_b = work.tile([P, NT], F32)
    nc.scalar.activation(p_b[:], mmb, mybir.ActivationFunctionType.Exp,
                         bias=negm[:], scale=1.0)

    s_b = work.tile([P, 1], F32)
    nc.vector.tensor_reduce(out=s_b[:], in_=p_b[:], axis=mybir.AxisListType.X,
                            op=mybir.AluOpType.add)
    s_all = work.tile([P, 1], F32)
    nc.gpsimd.partition_all_reduce(s_all[:], s_b[:], P, ReduceOp.add)
    r_sb = work.tile([P, 1], F32)
    nc.vector.reciprocal(r_sb[:], s_all[:])

    attn_b = work.tile([P, NT], F32)
    nc.vector.tensor_scalar(out=attn_b[:], in0=p_b[:], scalar1=r_sb[:],
                            scalar2=None, op0=mybir.AluOpType.mult)
    # out stays [p, lt]; the host does the 16 KB [p, lt] -> l reorder
    nc.scalar.dma_start(out_ext[b * P : (b + 1) * P, :], attn_b[:])
    return nd


def build_nc(repeat=1, use_for_i=False, tg=TG, inp_bufs=10,
             queues=("sync", "scalar"), mm_bufs=4):
    nc = bacc.Bacc(trn_type="TRN2")

    enc = nc.declare_dram_parameter("enc", [BS * H // tg, tg * L], F16,
                                    isOutput=False)
    hid = nc.declare_dram_parameter("hid", [P, HC * BS], F16, isOutput=False)
    out_ext = nc.declare_dram_parameter("out", [BS * P, NT], F32, isOutput=True)

    with tile.TileContext(nc) as tc:
        with (
            tc.tile_pool(name="consts", bufs=1) as consts,
            tc.tile_pool(name="inp", bufs=inp_bufs) as inp,
            tc.tile_pool(name="work", bufs=2) as work,
            tc.tile_pool(name="mmps", bufs=mm_bufs, space="PSUM") as mmps,
        ):
            hid_sb = consts.tile([P, HC * BS], F16)
            nc.sync.dma_start(hid_sb[:], hid[:])
            z16 = consts.tile([P, NT * BS], F16)
            nc.gpsimd.memset(z16[:], 0.0)

            def body():
                nd = 0
                for b in range(BS):
                    nd = batch_block(nc, inp, enc, hid_sb, z16, mmps,
                                     work, out_ext, b, tg, queues, nd)

            if use_for_i and repeat > 1:
                with tc.For_i(0, repeat, 1):
                    body()
            else:
                for _rep in range(repeat):
                    body()

    nc.compile()
    return nc


def make_in_maps(hidden, encoder_outputs, tg=TG):
    hidden = np.asarray(hidden)
    enc = np.asarray(encoder_outputs)
    assert hidden.shape == (H, B) and enc.shape == (L, B, H)

    enc16 = enc.astype(np.float16)
    hid16 = hidden.astype(np.float16)
    ngrp = HC // tg

    in_maps = []
    for c in range(NCORES):
        bsl = slice(c * BS, (c + 1) * BS)
        # [L, BS, H] -> [b, h, l] -> [b, g, p, t, l]: one contiguous
        # [P, tg*L] block per DMA, tg*8KB per-partition lines
        enc_t = np.ascontiguousarray(
            enc16[:, bsl, :]
            .transpose(1, 2, 0)
            .reshape(BS, ngrp, tg, P, L)
            .transpose(0, 1, 3, 2, 4)
        ).reshape(BS * H // tg, tg * L)
        # [H, BS] -> [hc, ph, b] -> [ph, hc*BS + b]
        hid_t = np.ascontiguousarray(
            hid16[:, bsl].reshape(HC, P, BS).transpose(1, 0, 2)
        ).reshape(P, HC * BS)
        in_maps.append({"enc": enc_t, "hid": hid_t})
    return in_maps


def assemble(res_list):
    """Per-core out is [(b, p), lt]; attn[b, l=lt*128+p] = out[b*128+p, lt]."""
    outs = []
    for r in res_list:
        a = r["out"].reshape(BS, P, NT).transpose(0, 2, 1).reshape(BS, 1, L)
        outs.append(a)
    return np.concatenate(outs, axis=0)


def _get_nc():
    if "nc" not in _cached:
        _cached["nc"] = build_nc()
    return _cached["nc"]


def kernel(hidden, encoder_outputs, **kwargs):
    in_maps = make_in_maps(hidden, encoder_outputs)
    nc = _get_nc()
    res = run_bass_kernel_spmd(nc, in_maps, core_ids=list(range(NCORES)))
    return assemble([res.results[i] for i in range(NCORES)])


# revision 17
# speedup vs baseline: 1.0632x; 1.0329x over previous
"""Bass kernel for nn_Attn_1898375545663 on 8 TRN2 NeuronCores.

Reference (single device):
    energies[b, l] = sum_h hidden[h, b] * encoder_outputs[l, b, h]   # [B, L]
    attn = softmax(energies, axis=1)                                 # [B, L]
    return attn[:, None, :]                                          # [B, 1, L]

Shapes: L=4096, B=32, H=1024. encoder_outputs is 512 MB fp32 -> memory bound.
Sharding: pure data parallel over batch; each of the 8 cores gets 4 batches,
no collectives.

vs the fp32 baseline (~200 us):

- The 2e-2 rel-err budget admits fp16 inputs (measured l2 rel err 1.9e-3,
  ~10x margin), halving HBM traffic to 32 MB/core -> ~94-100 us DMA floor.
- The custom-DVE reduce has no 16-bit fast path (would be DVE-bound at
  ~137 us), so the dot products run on the PE instead: the host
  pre-transposes each core's shard to [(b, g, ph), (t, l)] fp16 so every
  DMA is one contiguous 1 MB block ([128 partitions x 8 KB]); per
  (b, hc) chunk, 32 matmuls with stationary lhsT = enc[:, lt*128:+128]
  ([K=128 h, M=128 l]) and moving rhs = hid[:, hc*4:+4] ([128 h, 4 b])
  accumulate into a per-batch PSUM bank mm[128 l, 32*4]. fp16 stationary
  streams 1 col/cycle @ 2.4 GHz -> ~55 us PE busy, under the DMA floor.
- PSUM start=True marks the whole 2 KB bank pending-zero (writes to
  pending bytes store-and-clear, others accumulate), so each batch opens
  its bank with ONE full-width zeroing matmul and everything after
  accumulates with start=False; the full-width write also gives every
  later matmul a WAW dep on the reset (ordering).
- The per-batch softmax tail keeps the PE untouched (any PE op in the tail
  stalls the in-order PE stream behind DVE/ACT dependencies): max/sum
  partition reductions run on the otherwise-idle gpsimd
  (partition_all_reduce), exp reads the PSUM accumulator directly, the
  1/sum scale is one DVE tensor_scalar, and the [128 p, 32 lt] block DMAs
  out untransposed -- the host does the free 16 KB reorder. Tails for
  batches 0..2 hide under the next batch's stream; only the last ~3 us is
  exposed. mmps rotates over 4 full banks so bank reuse never couples the
  PE stream to an older batch's tail.
"""

import numpy as np

from concourse import bacc, mybir, tile
from concourse.bass_isa import ReduceOp
from concourse.bass_utils import run_bass_kernel_spmd

L, B, H = 4096, 32, 1024
NCORES = 8
BS = B // NCORES          # 4 batches per core
P = 128                   # partitions
HC = H // P               # 8 h-chunks per batch
NT = L // P               # 32 l-tiles per batch
TG = 1                    # h-chunks per DMA (1 MB per dma_start)
F32 = mybir.dt.float32
F16 = mybir.dt.float16

_cached = {}


def batch_block(nc, inp, enc, hid_sb, z16, mmps, work, out_ext,
                b, tg, queues, nd0, lt_step=1):
    """Stream + accumulate one batch's energies, then its softmax tail."""
    ngrp = HC // tg
    mm = mmps.tile([P, 512], F32)  # full PSUM bank (zero-region isolation)
    nc.tensor.matmul(mm[:, : NT * BS], z16[:], z16[:], start=True, stop=False)
    nd = nd0
    for g in range(ngrp):
        tile_in = inp.tile([P, tg * L], F16)
        r0 = (b * ngrp + g) * P
        eng = getattr(nc, queues[nd % len(queues)])
        nd += 1
        eng.dma_start(tile_in[:], enc[r0 : r0 + P, : tg * L])
        for t in range(tg):
            hc = g * tg + t
            for lt in range(0, NT, lt_step):
                nc.tensor.matmul(
                    mm[:, lt * BS : (lt + 1) * BS],
                    tile_in[:, t * L + lt * P : t * L + (lt + 1) * P],
                    hid_sb[:, hc * BS : (hc + 1) * BS],
                    start=False,
                    stop=(hc == HC - 1 and lt == NT - 1),
                )
    # column b of each [P, BS] group is this batch's energies:
    # mmb[p, lt] = e(l = lt*128 + p)
    mmb = mm[:, : NT * BS].rearrange("p (lt four) -> p lt four", four=BS)[:, :, b]

    # ---- softmax tail, PE-free (per-row max subtraction is exact) ----
    m1 = work.tile([P, 1], F32)
    nc.vector.tensor_reduce(out=m1[:], in_=mmb, axis=mybir.AxisListType.X,
                            op=mybir.AluOpType.max)
    mx = work.tile([P, 1], F32)
    nc.gpsimd.partition_all_reduce(mx[:], m1[:], P, ReduceOp.max)
    negm = work.tile([P, 1], F32)
    nc.scalar.mul(negm[:], mx[:], -1.0)

    p_b = work.tile([P, NT], F32)
    nc.scalar.activation(p_b[:], mmb, mybir.ActivationFunctionType.Exp,
                         bias=negm[:], scale=1.0)

    s_b = work.tile([P, 1], F32)
    nc.vector.tensor_reduce(out=s_b[:], in_=p_b[:], axis=mybir.AxisListType.X,
                            op=mybir.AluOpType.add)
    s_all = work.tile([P, 1], F32)
    nc.gpsimd.partition_all_reduce(s_all[:], s_b[:], P, ReduceOp.add)
    r_sb = work.tile([P, 1], F32)
    nc.vector.reciprocal(r_sb[:], s_all[:])

    attn_b = work.tile([P, NT], F32)
    nc.vector.tensor_scalar(out=attn_b[:], in0=p_b[:], scalar1=r_sb[:],
                            scalar2=None, op0=mybir.AluOpType.mult)
    # out stays [p, lt]; the host does the 16 KB [p, lt] -> l reorder
    nc.scalar.dma_start(out_ext[b * P : (b + 1) * P, :], attn_b[:])
    return nd


def build_nc(repeat=1, use_for_i=False, tg=TG, inp_bufs=10,
             queues=("sync", "scalar"), mm_bufs=4, lt_step=1):
    nc = bacc.Bacc(trn_type="TRN2")

    enc = nc.declare_dram_parameter("enc", [BS * H // tg, tg * L], F16,
                                    isOutput=False)
    hid = nc.declare_dram_parameter("hid", [P, HC * BS], F16, isOutput=False)
    out_ext = nc.declare_dram_parameter("out", [BS * P, NT], F32, isOutput=True)

    with tile.TileContext(nc) as tc:
        with (
            tc.tile_pool(name="consts", bufs=1) as consts,
            tc.tile_pool(name="inp", bufs=inp_bufs) as inp,
            tc.tile_pool(name="work", bufs=2) as work,
            tc.tile_pool(name="mmps", bufs=mm_bufs, space="PSUM") as mmps,
        ):
            hid_sb = consts.tile([P, HC * BS], F16)
            nc.sync.dma_start(hid_sb[:], hid[:])
            z16 = consts.tile([P, NT * BS], F16)
            nc.gpsimd.memset(z16[:], 0.0)

            def body():
                nd = 0
                for b in range(BS):
                    nd = batch_block(nc, inp, enc, hid_sb, z16, mmps,
                                     work, out_ext, b, tg, queues, nd,
                                     lt_step=lt_step)

            if use_for_i and repeat > 1:
                with tc.For_i(0, repeat, 1):
                    body()
            else:
                for _rep in range(repeat):
                    body()

    nc.compile()
    return nc


def make_in_maps(hidden, encoder_outputs, tg=TG):
    hidden = np.asarray(hidden)
    enc = np.asarray(encoder_outputs)
    assert hidden.shape == (H, B) and enc.shape == (L, B, H)

    enc16 = enc.astype(np.float16)
    hid16 = hidden.astype(np.float16)
    ngrp = HC // tg

    in_maps = []
    for c in range(NCORES):
        bsl = slice(c * BS, (c + 1) * BS)
        # [L, BS, H] -> [b, h, l] -> [b, g, p, t, l]: one contiguous
        # [P, tg*L] block per DMA, tg*8KB per-partition lines
        enc_t = np.ascontiguousarray(
            enc16[:, bsl, :]
            .transpose(1, 2, 0)
            .reshape(BS, ngrp, tg, P, L)
            .transpose(0, 1, 3, 2, 4)
        ).reshape(BS * H // tg, tg * L)
        # [H, BS] -> [hc, ph, b] -> [ph, hc*BS + b]
        hid_t = np.ascontiguousarray(
            hid16[:, bsl].reshape(HC, P, BS).transpose(1, 0, 2)
        ).reshape(P, HC * BS)
        in_maps.append({"enc": enc_t, "hid": hid_t})
    return in_maps


def assemble(res_list):
    """Per-core out is [(b, p), lt]; attn[b, l=lt*128+p] = out[b*128+p, lt]."""
    outs = []
    for r in res_list:
        a = r["out"].reshape(BS, P, NT).transpose(0, 2, 1).reshape(BS, 1, L)
        outs.append(a)
    return np.concatenate(outs, axis=0)


def _get_nc():
    if "nc" not in _cached:
        _cached["nc"] = build_nc()
    return _cached["nc"]


def kernel(hidden, encoder_outputs, **kwargs):
    in_maps = make_in_maps(hidden, encoder_outputs)
    nc = _get_nc()
    res = run_bass_kernel_spmd(nc, in_maps, core_ids=list(range(NCORES)))
    return assemble([res.results[i] for i in range(NCORES)])
